# revision 1
# baseline (speedup 1.0000x reference)
"""TRN2 Bass kernel for nn_Attention_5720896438407 (8-core data-parallel).

Mathematical collapse: the module computes SDPA over the *head* axis with a
single KV head (KV=1), so the softmax runs over a size-1 axis and every
attention weight is exactly 1.0.  The q path (q_a/q_norm/q_b), both rotary
embeddings, the nope/rope blend and the attention mask all cancel out, and
the module reduces to

    T  = hidden @ kv_a_w.T + kv_a_b                    # (ntok, 512)
    s  = rsqrt(mean(T^2, -1) + eps)                    # per-token RMS scale
    V  = (s*T) @ (kv_b_w[128:] * (1 + kv_norm_w)).T + kv_b_b[128:]
    Y  = V @ M.T      with  M = o_w.reshape(2048, 16, 128).sum(1)

(the attention output tiles V across all 16 heads, so o_proj sees the head
sum of its weight).  This is what the kernel computes, numerically verified
to ~3e-7 relative error against the full reference in fp64.

Distribution: pure data-parallel over the 8192 tokens — 1024 tokens per
NeuronCore, no collectives.  Per core the tokens stream through in 8 slabs
of 128 tokens; each slab's full pipeline (step-1 matmul, RMS, PE transpose,
V, Y, output DMA) starts as soon as its 0.5 MB input slab lands, so the
input stream (SP HWDGE ring) and output stream (ACT HWDGE ring) overlap for
the whole kernel.  Step-1 operands are fp16 (halves the dominant input
bytes; fp32 PSUM accumulation), the RMS statistics are computed in fp32,
the small downstream matmuls run in fp16, and Y ships as fp16 (host
casts back to fp32), halving the output stream.  End-to-end error vs the
fp32 reference is ~5e-4 relative.
"""
import sys

sys.path.insert(0, "/opt/trn_rl_repo")

import numpy as np
import concourse.bass as bass
import concourse.tile as tile
from concourse import bacc, mybir
from concourse.bass_utils import run_bass_kernel_spmd
from concourse.masks import make_identity

F32 = mybir.dt.float32
F16 = mybir.dt.float16

HID = 2048
KV = 512
D = 128
OUT = 2048
EPS = 1e-6
N_HID_CK = HID // 128   # 16
N_KV_CK = KV // 128     # 4
N_OUT_T = OUT // 512    # 4
SLAB = 128              # tokens per slab
N_CORES = 8
AF = mybir.ActivationFunctionType

_NC_CACHE = {}


def _build_nc(tok, with_ba):
    nslab = tok // SLAB
    assert tok % SLAB == 0

    nc = bacc.Bacc("TRN2", target_bir_lowering=False, debug=False,
                   num_devices=1)

    xts_d = nc.dram_tensor("xts", (nslab, 128, N_HID_CK, SLAB), F16,
                           kind="ExternalInput").ap()
    w1s_d = nc.dram_tensor("w1s", (128, N_HID_CK, KV), F16,
                           kind="ExternalInput").ap()
    wvt_d = nc.dram_tensor("wvt", (KV, D), F16, kind="ExternalInput").ap()
    mt_d = nc.dram_tensor("mt", (D, OUT), F16, kind="ExternalInput").ap()
    bv_d = nc.dram_tensor("bv", (D, 1), F32, kind="ExternalInput").ap()
    if with_ba:
        ba_d = nc.dram_tensor("bar", (1, KV), F16, kind="ExternalInput").ap()
        onesr_d = nc.dram_tensor("onesr", (1, 128), F16,
                                 kind="ExternalInput").ap()
    y_d = nc.dram_tensor("y", (tok, OUT), F16, kind="ExternalOutput").ap()

    with tile.TileContext(nc) as tc:
        with tc.tile_pool(name="consts", bufs=1) as consts, \
             tc.tile_pool(name="slabs", bufs=8) as slabs, \
             tc.tile_pool(name="work", bufs=2) as work, \
             tc.tile_pool(name="ps_t", bufs=3, space="PSUM") as ps_t, \
             tc.tile_pool(name="ps_r", bufs=2, space="PSUM") as ps_r, \
             tc.tile_pool(name="ps_v", bufs=1, space="PSUM") as ps_v, \
             tc.tile_pool(name="ps_y", bufs=2, space="PSUM") as ps_y:
            # ---- input stream on the SP ring, in priority order:
            #      W1 quarters interleaved with slab-0 quarters, then the
            #      remaining slabs (each 0.5 MB, fully contiguous thanks to
            #      the host-side swizzle) ----
            w1_s = consts.tile([128, N_HID_CK, KV], F16, tag="w1")
            sg0 = slabs.tile([128, N_HID_CK, SLAB], F16, tag="slab",
                             name="slab0")
            for h in range(4):
                nc.sync.dma_start(w1_s[:, 4 * h:4 * h + 4, :],
                                  w1s_d[:, 4 * h:4 * h + 4, :])
                nc.sync.dma_start(sg0[:, 4 * h:4 * h + 4, :],
                                  xts_d[0, :, 4 * h:4 * h + 4, :])
            sg = [sg0]
            for g in range(1, nslab):
                t = slabs.tile([128, N_HID_CK, SLAB], F16, tag="slab",
                               name=f"slab{g}")
                nc.sync.dma_start(t[:], xts_d[g])
                sg.append(t)
            # ---- small constants + all output DMAs on the ACT ring ----
            wv_s = []
            for c in range(N_KV_CK):
                t = consts.tile([128, D], F16, tag=f"wv_{c}", name=f"wv_{c}")
                nc.scalar.dma_start(t[:], wvt_d[c * 128:(c + 1) * 128, :])
                wv_s.append(t)
            mt_s = consts.tile([128, OUT], F16, tag="mt")
            nc.scalar.dma_start(mt_s[:], mt_d)
            bv_s = consts.tile([128, 1], F32, tag="bv")
            nc.scalar.dma_start(bv_s[:], bv_d)
            if with_ba:
                ba_s = consts.tile([1, KV], F16, tag="ba")
                nc.scalar.dma_start(ba_s[:], ba_d)
                onesr_s = consts.tile([1, 128], F16, tag="onesr")
                nc.scalar.dma_start(onesr_s[:], onesr_d)
            # ---- PE warm-up: junk matmuls on the (early-ready) identity
            #      keep the HAM activity monitor from throttling the PE
            #      while the first data DMAs are in flight ----
            ident = consts.tile([128, 128], F16, tag="ident")
            make_identity(nc, ident[:])
            js = consts.tile([128, 512], F16, tag="js")
            nc.gpsimd.memset(js[:], 0.0)
            junka = ps_y.tile([128, 512], F32, tag="py", name="junka")
            junkb = ps_y.tile([128, 512], F32, tag="py", name="junkb")
            for i in range(10):
                nc.tensor.matmul(junka[:] if i % 2 == 0 else junkb[:],
                                 ident[:], js[:], start=True, stop=True)
            eps_s = consts.tile([128, 1], F32, tag="eps")
            nc.vector.memset(eps_s[:], EPS)

            def step1(g):
                # T.T slab accumulation, token-major: 16 chunk matmuls,
                # fp16 operands, fp32 PSUM.
                pt = ps_t.tile([128, KV], F32, tag="pt", name=f"pt{g}")
                for ck in range(N_HID_CK):
                    nc.tensor.matmul(
                        pt[:], sg[g][:, ck, :], w1_s[:, ck, :],
                        start=(ck == 0),
                        stop=(ck == N_HID_CK - 1 and not with_ba),
                    )
                if with_ba:
                    # rank-1 row-broadcast of kv_a_b into the accumulation
                    nc.tensor.matmul(pt[:], onesr_s[:], ba_s[:],
                                     start=False, stop=True)
                return pt

            def tail(g, pt):
                t0 = g * SLAB
                # RMS statistics: Square activation with free-axis
                # accumulator gives sum(T^2) per token in one op.
                sqj = work.tile([128, KV], F32, tag="sqj")
                ssq = work.tile([128, 1], F32, tag="ssq")
                nc.scalar.activation(sqj[:], pt[:], AF.Square,
                                     accum_out=ssq[:])
                rt = work.tile([128, 1], F32, tag="rt")
                nc.scalar.activation(rt[:], ssq[:], AF.Sqrt,
                                     bias=eps_s[:], scale=1.0 / KV)
                sc = work.tile([128, 1], F32, tag="sc")
                nc.vector.reciprocal(sc[:], rt[:])
                ttn = work.tile([128, KV], F16, tag="ttn", bufs=3)
                nc.vector.tensor_scalar_mul(ttn[:], pt[:], sc[:])
                # transpose the scaled T into kv-major for step 2
                trp = ps_r.tile([128, N_KV_CK, SLAB], F16, tag="trp",
                                name=f"trp{g}")
                for c in range(N_KV_CK):
                    nc.tensor.transpose(trp[:, c, :],
                                        ttn[:, c * 128:(c + 1) * 128],
                                        ident[:])
                ttr = work.tile([128, N_KV_CK, SLAB], F16, tag="ttr", bufs=3)
                nc.vector.tensor_copy(ttr[:], trp[:])
                # step 2: V.T = Wv' @ (sT).T, bias kv_b_b on the copy
                vtp = ps_v.tile([128, SLAB], F32, tag="vtp", name=f"vtp{g}")
                for c in range(N_KV_CK):
                    nc.tensor.matmul(vtp[:], wv_s[c][:], ttr[:, c, :],
                                     start=(c == 0),
                                     stop=(c == N_KV_CK - 1))
                vts = work.tile([128, SLAB], F16, tag="vts", bufs=3)
                nc.scalar.activation(vts[:], vtp[:], AF.Identity,
                                     bias=bv_s[:], scale=1.0)
                # step 4: Y = V @ M.T, plain PSUM->SBUF copies, 1 MB DMA out
                ysb = work.tile([128, OUT], F16, tag="ysb", bufs=6)
                for n in range(N_OUT_T):
                    py = ps_y.tile([128, 512], F32, tag="py",
                                   name=f"py{g}_{n}")
                    nc.tensor.matmul(py[:], vts[:],
                                     mt_s[:, n * 512:(n + 1) * 512],
                                     start=True, stop=True)
                    ysl = ysb[:, n * 512:(n + 1) * 512]
                    if n % 2 == 0:
                        nc.vector.tensor_copy(ysl, py[:])
                    else:
                        nc.scalar.activation(ysl, py[:], AF.Copy,
                                             bias=0.0, scale=1.0)
                    if g == nslab - 1 and n == 1:
                        # final slab: overlap the first output half with the
                        # remaining matmuls/copies so only 0.25 MB trails
                        nc.scalar.dma_start(y_d[t0:t0 + SLAB, 0:1024],
                                            ysb[:, 0:1024])
                if g == nslab - 1:
                    nc.scalar.dma_start(y_d[t0:t0 + SLAB, 1024:2048],
                                        ysb[:, 1024:2048])
                else:
                    nc.scalar.dma_start(y_d[t0:t0 + SLAB, :], ysb[:])

            # 2-stage software pipeline: slab g's tail is emitted after slab
            # g+1's step-1 matmuls so the PE never waits on the RMS chain.
            prev = None
            for g in range(nslab):
                pt = step1(g)
                if prev is not None:
                    tail(*prev)
                prev = (g, pt)
            tail(*prev)

    nc.compile()
    return nc


def _host_prep(inputs):
    """Fold weights, swizzle X into fp16 token slabs, shard across cores."""
    h = np.asarray(inputs["hidden_states"], dtype=np.float32)
    b, s, hid = h.shape
    assert hid == HID
    x = np.ascontiguousarray(h.reshape(b * s, hid))
    ntok = b * s
    tok = ntok // N_CORES
    nslab = tok // SLAB

    kv_a_w = np.asarray(inputs["kv_a_w"], np.float32)
    kv_a_b = np.asarray(inputs["kv_a_b"], np.float32)
    kv_norm_w = np.asarray(inputs["kv_norm_w"], np.float32)
    kv_b_w = np.asarray(inputs["kv_b_w"], np.float32)
    kv_b_b = np.asarray(inputs["kv_b_b"], np.float32)
    o_w = np.asarray(inputs["o_w"], np.float32)

    w1s = np.ascontiguousarray(
        kv_a_w.T.reshape(N_HID_CK, 128, KV).transpose(1, 0, 2)
    ).astype(np.float16)
    wv = kv_b_w[D:2 * D] * (1.0 + kv_norm_w)[None, :]
    wvt = np.ascontiguousarray(wv.T).astype(np.float16)
    M = o_w.reshape(HID, 16, D).sum(axis=1)
    mt = np.ascontiguousarray(M.T).astype(np.float16)
    bv = np.ascontiguousarray(kv_b_b[D:2 * D].reshape(D, 1)).astype(np.float32)
    with_ba = bool(np.any(kv_a_b != 0.0))
    ba_row = np.ascontiguousarray(kv_a_b.reshape(1, KV)).astype(np.float16)
    ones_row = np.ones((1, 128), np.float16)

    in_maps = []
    for i in range(N_CORES):
        shard = x[i * tok:(i + 1) * tok]
        xts = np.ascontiguousarray(
            shard.T.reshape(N_HID_CK, 128, nslab, SLAB).transpose(2, 1, 0, 3)
        ).astype(np.float16)
        m = {"xts": xts, "w1s": w1s, "wvt": wvt, "mt": mt, "bv": bv}
        if with_ba:
            m["bar"] = ba_row
            m["onesr"] = ones_row
        in_maps.append(m)

    def gather(results):
        y = np.concatenate([r["y"] for r in results], axis=0)
        return np.ascontiguousarray(y.reshape(b, s, HID).astype(np.float32))

    return in_maps, gather, with_ba, tok


def _run(inputs, trace=False, **spmd_kwargs):
    in_maps, gather, with_ba, tok = _host_prep(inputs)
    key = (tok, with_ba)
    if key not in _NC_CACHE:
        _NC_CACHE[key] = _build_nc(tok, with_ba)
    nc = _NC_CACHE[key]
    res = run_bass_kernel_spmd(nc, in_maps, core_ids=list(range(N_CORES)),
                               trace=trace, **spmd_kwargs)
    return gather(res.results), res


def kernel(**inputs) -> np.ndarray:
    y, _ = _run(inputs, trace=False)
    return y



# revision 2
# speedup vs baseline: 1.0082x; 1.0082x over previous
"""TRN2 Bass kernel for nn_Attention_5720896438407 (8-core data-parallel), v7.

Math (see v2 docstring): attention collapses to
    Y = s * (x @ Wf.T @ M.T) + bias-terms,   s = rsqrt(mean(T^2) + eps),
    T = x @ kv_a_w.T   (needed only for stats -> fp8 DoubleRow at 2x rate)
with Wf = (kv_b_w[128:]*(1+kv_norm_w)) @ kv_a_w and M = head-sum of o_w,
both folded on the host.

Scheduling notes (hard-won):
- ~7us fixed framework preamble before any sequencer can issue; each
  dma_start costs its sequencer ~0.65us (DIRECT2D), so DMA issues are a
  scarce resource.  All input flows on the sync ring in strict priority
  order (the single ring naturally prioritizes early pieces); wf/mt ride
  the scalar ring before ACT work starts; outputs ride the gpsimd SWDGE
  ring (slow, but each output is only 0.5 MiB and off the critical path).
- First stats matmul gates on just 160 KiB (w8[0:2] + x8 slab0[0:2]).
- The PE executes strictly in emission order, so every emission is placed
  after its deps (dma piece, vts copy, s chain) are already met; vraw's 16
  matmuls are emitted in two halves so the second MiB of x16 never blocks
  stats matmuls behind it.
- 8 small junk matmuls on memset tiles bridge the preamble->data window to
  keep the HAM clock gate from oscillating.
"""
import sys

sys.path.insert(0, "/opt/trn_rl_repo")

import numpy as np
import ml_dtypes
import concourse.bass as bass
import concourse.tile as tile
from concourse import bacc, mybir
from concourse.bass_utils import run_bass_kernel_spmd

F32 = mybir.dt.float32
F16 = mybir.dt.float16
F8 = mybir.dt.float8e4
DR = mybir.MatmulPerfMode.DoubleRow
AF = mybir.ActivationFunctionType

HID = 2048
KV = 512
D = 128
OUT = 2048
EPS = 1e-6
NCK = HID // 128         # 16 hid chunks
SLAB = 128               # tokens per stats slab
VGRP = 512               # tokens per value-path group
N_CORES = 8
WSCALE = 64.0            # host pre-scale on kv_a_w so fp8 stays normal

_NC_CACHE = {}


def _build_nc(tok, with_ba, with_bv):
    nslab = tok // SLAB        # 8
    ngrp = tok // VGRP         # 2
    assert tok % VGRP == 0 and VGRP % SLAB == 0

    nc = bacc.Bacc("TRN2", target_bir_lowering=False, debug=False,
                   num_devices=1)

    x8_d = nc.dram_tensor("x8", (nslab // 2, 128, 2, NCK, SLAB), F8,
                          kind="ExternalInput").ap()
    x16_d = nc.dram_tensor("x16", (ngrp, 128, NCK, VGRP), F16,
                           kind="ExternalInput").ap()
    w8_d = nc.dram_tensor("w8", (128, NCK, KV), F8, kind="ExternalInput").ap()
    wf_d = nc.dram_tensor("wf", (128, NCK, D), F16, kind="ExternalInput").ap()
    mt_d = nc.dram_tensor("mt", (D, OUT), F16, kind="ExternalInput").ap()
    if with_ba:
        ba_d = nc.dram_tensor("bar", (1, KV), F8, kind="ExternalInput").ap()
        ones8_d = nc.dram_tensor("ones8", (1, 128), F8,
                                 kind="ExternalInput").ap()
        vb_d = nc.dram_tensor("vb", (128, 1), F32, kind="ExternalInput").ap()
    if with_bv:
        crb_d = nc.dram_tensor("crb", (128, OUT), F16,
                               kind="ExternalInput").ap()
    y_d = nc.dram_tensor("y", (tok, OUT), F16, kind="ExternalOutput").ap()

    with tile.TileContext(nc) as tc:
        with tc.tile_pool(name="consts", bufs=1) as consts, \
             tc.tile_pool(name="xs8", bufs=1) as xs8, \
             tc.tile_pool(name="xs16", bufs=1) as xs16, \
             tc.tile_pool(name="work", bufs=2) as work, \
             tc.tile_pool(name="ps_t", bufs=2, space="PSUM") as ps_t, \
             tc.tile_pool(name="ps_v", bufs=2, space="PSUM") as ps_v, \
             tc.tile_pool(name="ps_y", bufs=3, space="PSUM") as ps_y:
            w8_s = consts.tile([128, NCK, KV], F8, tag="w8")
            x8_s = xs8.tile([128, nslab, NCK, SLAB], F8, tag="x8")
            x16_s = [xs16.tile([128, NCK, VGRP], F16, tag=f"x16_{q}",
                               name=f"x16_{q}") for q in range(ngrp)]
            # sync ring: strict priority order, tiny leading pieces
            # pieces sized for >=4KB per partition line: DMA descriptors
            # are per line, so sub-KB lines are descriptor-rate-bound
            nc.sync.dma_start(w8_s[:, 0:8, :], w8_d[:, 0:8, :])
            nc.sync.dma_start(x8_s[:, 0:2, :, :], x8_d[0])
            nc.sync.dma_start(w8_s[:, 8:16, :], w8_d[:, 8:16, :])
            nc.sync.dma_start(x16_s[0][:, 0:8, :], x16_d[0, :, 0:8, :])
            nc.sync.dma_start(x8_s[:, 2:4, :, :], x8_d[1])
            nc.sync.dma_start(x16_s[0][:, 8:16, :], x16_d[0, :, 8:16, :])
            nc.sync.dma_start(x8_s[:, 4:6, :, :], x8_d[2])
            nc.sync.dma_start(x16_s[1][:, 0:8, :], x16_d[1, :, 0:8, :])
            nc.sync.dma_start(x8_s[:, 6:8, :, :], x8_d[3])
            nc.sync.dma_start(x16_s[1][:, 8:16, :], x16_d[1, :, 8:16, :])
            # scalar ring: wf/mt issue AFTER the activation-table preloads
            # below, keeping the first ~3us of DMA bandwidth for the
            # stats-critical sync-ring pieces
            wf_s = consts.tile([128, NCK, D], F16, tag="wf")
            mt_s = consts.tile([128, OUT], F16, tag="mt")
            if with_ba:
                ba_s = consts.tile([1, KV], F8, tag="ba")
                nc.scalar.dma_start(ba_s[:], ba_d)
                ones8_s = consts.tile([1, 128], F8, tag="ones8")
                nc.scalar.dma_start(ones8_s[:], ones8_d)
                vb_s = consts.tile([128, 1], F32, tag="vb")
                nc.scalar.dma_start(vb_s[:], vb_d)
            if with_bv:
                crb_s = consts.tile([128, OUT], F16, tag="crb")
                nc.scalar.dma_start(crb_s[:], crb_d)

            # ---------------- local consts + PE warm-up ----------------
            eps_s = consts.tile([128, 1], F32, tag="eps")
            nc.vector.memset(eps_s[:], EPS)
            jsa = consts.tile([128, 128], F16, tag="jsa")
            nc.vector.memset(jsa[:], 0.0)
            jsb = consts.tile([128, 512], F16, tag="jsb")
            nc.vector.memset(jsb[:], 0.0)
            # preload Square/Sqrt activation tables off the critical path
            tl = consts.tile([128, 1], F32, tag="tl")
            nc.scalar.activation(tl[:], eps_s[:], AF.Square)
            nc.scalar.activation(tl[:], eps_s[:], AF.Sqrt)
            nc.scalar.dma_start(wf_s[:], wf_d)
            nc.scalar.dma_start(mt_s[:], mt_d)
            for i in range(8):
                junk = ps_y.tile([128, 512], F32, tag="py", name=f"junk{i}")
                nc.tensor.matmul(junk[:, 0:256], jsa[:], jsb[:, 0:256],
                                 start=True, stop=True)

            # ---------------- per-stage bodies ----------------
            sq_sc = work.tile([128, KV], F16, tag="sq", bufs=2)
            vts = consts.tile([128, tok], F16, tag="vts")
            s_t = [None] * nslab
            pv_t = [None] * ngrp

            def stats(g):
                pt = ps_t.tile([128, KV], F32, tag="pt", name=f"pt{g}")
                for k in range(8):
                    nc.tensor.matmul(pt[:], x8_s[:, g, 2 * k:2 * k + 2, :],
                                     w8_s[:, 2 * k:2 * k + 2, :],
                                     start=(k == 0),
                                     stop=(k == 7 and not with_ba),
                                     perf_mode=DR)
                if with_ba:
                    nc.tensor.matmul(pt[:], ones8_s[:], ba_s[:],
                                     start=False, stop=True)
                ssq = work.tile([128, 1], F32, tag="ssq")
                nc.scalar.activation(sq_sc[:], pt[:], AF.Square,
                                     accum_out=ssq[:])
                rt = work.tile([128, 1], F32, tag="rt")
                nc.scalar.activation(rt[:], ssq[:], AF.Sqrt, bias=eps_s[:],
                                     scale=1.0 / (KV * WSCALE * WSCALE))
                sg = work.tile([128, 1], F32, tag="sg", name=f"sg{g}", bufs=4)
                nc.vector.reciprocal(sg[:], rt[:])
                s_t[g] = sg

            def vraw_a(q):
                pv = ps_v.tile([128, VGRP], F32, tag="pv", name=f"pv{q}")
                pv_t[q] = pv
                for ck in range(8):
                    nc.tensor.matmul(pv[:], wf_s[:, ck, :], x16_s[q][:, ck, :],
                                     start=(ck == 0), stop=False)

            def vraw_b(q):
                pv = pv_t[q]
                for ck in range(8, NCK):
                    nc.tensor.matmul(pv[:], wf_s[:, ck, :], x16_s[q][:, ck, :],
                                     start=False, stop=(ck == NCK - 1))
                dst = vts[:, q * VGRP:(q + 1) * VGRP]
                if with_ba:
                    nc.scalar.activation(dst, pv[:], AF.Identity,
                                         bias=vb_s[:], scale=1.0)
                else:
                    nc.vector.tensor_copy(dst, pv[:])

            def step4(g):
                t0 = g * SLAB
                ysb = work.tile([128, OUT], F16, tag="ysb", bufs=6)
                for n in range(4):
                    py = ps_y.tile([128, 512], F32, tag="py",
                                   name=f"py{g}_{n}")
                    nc.tensor.matmul(py[:], vts[:, t0:t0 + SLAB],
                                     mt_s[:, n * 512:(n + 1) * 512],
                                     start=True, stop=True)
                    ysl = ysb[:, n * 512:(n + 1) * 512]
                    if n % 2 == 0:
                        nc.vector.tensor_scalar_mul(ysl, py[:], s_t[g][:])
                    else:
                        nc.scalar.activation(ysl, py[:], AF.Identity,
                                             bias=0.0, scale=s_t[g][:])
                    if with_bv:
                        nc.vector.tensor_add(
                            ysl, ysl, crb_s[:, n * 512:(n + 1) * 512])
                    if g == nslab - 1 and n == 1:
                        nc.sync.dma_start(y_d[t0:t0 + SLAB, 0:1024],
                                          ysb[:, 0:1024])
                # alternate output rings: sync's HWDGE is idle once the input
                # issues drain, and two rings halve the serial issue cost
                ring = nc.gpsimd if g % 2 == 0 else nc.sync
                if g == nslab - 1:
                    ring.dma_start(y_d[t0:t0 + SLAB, 1024:2048],
                                   ysb[:, 1024:2048])
                else:
                    ring.dma_start(y_d[t0:t0 + SLAB, :], ysb[:])

            # ---------------- PE emission order ----------------
            stats(0)
            stats(1)
            vraw_a(0)
            vraw_b(0)
            stats(2)
            step4(0)
            stats(3)
            step4(1)
            vraw_a(1)
            stats(4)
            step4(2)
            vraw_b(1)
            stats(5)
            step4(3)
            stats(6)
            step4(4)
            stats(7)
            step4(5)
            step4(6)
            step4(7)

    nc.compile()
    return nc


def _host_prep(inputs):
    h = np.asarray(inputs["hidden_states"], dtype=np.float32)
    b, s, hid = h.shape
    assert hid == HID
    x = np.ascontiguousarray(h.reshape(b * s, hid))
    ntok = b * s
    tok = ntok // N_CORES
    nslab = tok // SLAB
    ngrp = tok // VGRP

    kv_a_w = np.asarray(inputs["kv_a_w"], np.float64)
    kv_a_b = np.asarray(inputs["kv_a_b"], np.float64)
    kv_norm_w = np.asarray(inputs["kv_norm_w"], np.float64)
    kv_b_w = np.asarray(inputs["kv_b_w"], np.float64)
    kv_b_b = np.asarray(inputs["kv_b_b"], np.float64)
    o_w = np.asarray(inputs["o_w"], np.float64)

    wv = kv_b_w[D:2 * D] * (1.0 + kv_norm_w)[None, :]          # (128, 512)
    wf = wv @ kv_a_w                                           # (128, 2048)
    M = o_w.reshape(HID, 16, D).sum(axis=1)                    # (2048, 128)

    w8 = np.ascontiguousarray(
        (kv_a_w.T * WSCALE).reshape(NCK, 128, KV).transpose(1, 0, 2)
    ).astype(np.float32).astype(ml_dtypes.float8_e4m3)
    wf_sw = np.ascontiguousarray(
        wf.T.reshape(NCK, 128, D).transpose(1, 0, 2)).astype(np.float16)
    mt = np.ascontiguousarray(M.T).astype(np.float16)

    with_ba = bool(np.any(kv_a_b != 0.0))
    with_bv = bool(np.any(kv_b_b[D:2 * D] != 0.0))

    in_maps = []
    for i in range(N_CORES):
        shard = x[i * tok:(i + 1) * tok]                       # (tok, 2048)
        xt = shard.T.reshape(NCK, 128, tok)                    # hid-major
        x8 = np.ascontiguousarray(
            xt.reshape(NCK, 128, nslab // 2, 2, SLAB).transpose(2, 1, 3, 0, 4)
        ).astype(ml_dtypes.float8_e4m3)
        x16 = np.ascontiguousarray(
            xt.reshape(NCK, 128, ngrp, VGRP).transpose(2, 1, 0, 3)
        ).astype(np.float16)
        m = {"x8": x8, "x16": x16, "w8": w8, "wf": wf_sw, "mt": mt}
        if with_ba:
            m["bar"] = (kv_a_b.reshape(1, KV) * WSCALE).astype(
                np.float32).astype(ml_dtypes.float8_e4m3)
            m["ones8"] = np.ones((1, 128), np.float32).astype(
                ml_dtypes.float8_e4m3)
            m["vb"] = np.ascontiguousarray(
                (wv @ kv_a_b).reshape(D, 1)).astype(np.float32)
        if with_bv:
            cr = (M @ kv_b_b[D:2 * D]).reshape(1, OUT)
            m["crb"] = np.ascontiguousarray(
                np.broadcast_to(cr, (128, OUT))).astype(np.float16)
        in_maps.append(m)

    def gather(results):
        y = np.concatenate([r["y"] for r in results], axis=0)
        return np.ascontiguousarray(y.reshape(b, s, HID).astype(np.float32))

    return in_maps, gather, with_ba, with_bv, tok


def _run(inputs, trace=False, **spmd_kwargs):
    in_maps, gather, with_ba, with_bv, tok = _host_prep(inputs)
    key = (tok, with_ba, with_bv)
    if key not in _NC_CACHE:
        _NC_CACHE[key] = _build_nc(tok, with_ba, with_bv)
    nc = _NC_CACHE[key]
    res = run_bass_kernel_spmd(nc, in_maps, core_ids=list(range(N_CORES)),
                               trace=trace, **spmd_kwargs)
    return gather(res.results), res


def kernel(**inputs) -> np.ndarray:
    y, _ = _run(inputs, trace=False)
    return y


# revision 5
# speedup vs baseline: 1.0167x; 1.0084x over previous
"""TRN2 Bass kernel for nn_Attention_5720896438407 (8-core data-parallel), v7.

Math (see v2 docstring): attention collapses to
    Y = s * (x @ Wf.T @ M.T) + bias-terms,   s = rsqrt(mean(T^2) + eps),
    T = x @ kv_a_w.T   (needed only for stats -> fp8 DoubleRow at 2x rate)
with Wf = (kv_b_w[128:]*(1+kv_norm_w)) @ kv_a_w and M = head-sum of o_w,
both folded on the host.

Scheduling notes (hard-won):
- ~7us fixed framework preamble before any sequencer can issue; each
  dma_start costs its sequencer ~0.65us (DIRECT2D), so DMA issues are a
  scarce resource.  All input flows on the sync ring in strict priority
  order (the single ring naturally prioritizes early pieces); wf/mt ride
  the scalar ring before ACT work starts; outputs ride the gpsimd SWDGE
  ring (slow, but each output is only 0.5 MiB and off the critical path).
- First stats matmul gates on 0.75 MiB (w8 quarter + x8 pair-block).
- The PE executes strictly in emission order, so every emission is placed
  after its deps (dma piece, vts copy, s chain) are already met; vraw's 16
  matmuls are emitted in two halves so the second MiB of x16 never blocks
  stats matmuls behind it.
- 8 small junk matmuls on memset tiles bridge the preamble->data window to
  keep the HAM clock gate from oscillating.
"""
import sys

sys.path.insert(0, "/opt/trn_rl_repo")

import numpy as np
import ml_dtypes
import concourse.bass as bass
import concourse.tile as tile
from concourse import bacc, mybir
from concourse.bass_utils import run_bass_kernel_spmd

F32 = mybir.dt.float32
F16 = mybir.dt.float16
F8 = mybir.dt.float8e4
DR = mybir.MatmulPerfMode.DoubleRow
AF = mybir.ActivationFunctionType

HID = 2048
KV = 512
D = 128
OUT = 2048
EPS = 1e-6
NCK = HID // 128         # 16 hid chunks
SLAB = 128               # tokens per stats slab
VGRP = 512               # tokens per value-path group
N_CORES = 8
WSCALE = 64.0            # host pre-scale on kv_a_w so fp8 stays normal

_NC_CACHE = {}


def _build_nc(tok, with_ba, with_bv):
    nslab = tok // SLAB        # 8
    ngrp = tok // VGRP         # 2
    assert tok % VGRP == 0 and VGRP % SLAB == 0

    nc = bacc.Bacc("TRN2", target_bir_lowering=False, debug=False,
                   num_devices=1)

    x8_d = nc.dram_tensor("x8", (nslab // 2, 128, 2, NCK, SLAB), F8,
                          kind="ExternalInput").ap()
    x16_d = nc.dram_tensor("x16", (ngrp, 128, NCK, VGRP), F16,
                           kind="ExternalInput").ap()
    w8_d = nc.dram_tensor("w8", (128, NCK, KV), F8, kind="ExternalInput").ap()
    wf_d = nc.dram_tensor("wf", (128, NCK, D), F16, kind="ExternalInput").ap()
    mt_d = nc.dram_tensor("mt", (D, OUT), F16, kind="ExternalInput").ap()
    if with_ba:
        ba_d = nc.dram_tensor("bar", (1, KV), F8, kind="ExternalInput").ap()
        ones8_d = nc.dram_tensor("ones8", (1, 128), F8,
                                 kind="ExternalInput").ap()
        vb_d = nc.dram_tensor("vb", (128, 1), F32, kind="ExternalInput").ap()
    if with_bv:
        crb_d = nc.dram_tensor("crb", (128, OUT), F16,
                               kind="ExternalInput").ap()
    y_d = nc.dram_tensor("y", (tok, OUT), F16, kind="ExternalOutput").ap()

    with tile.TileContext(nc) as tc:
        with tc.tile_pool(name="consts", bufs=1) as consts, \
             tc.tile_pool(name="xs8", bufs=1) as xs8, \
             tc.tile_pool(name="xs16", bufs=1) as xs16, \
             tc.tile_pool(name="work", bufs=2) as work, \
             tc.tile_pool(name="ps_t", bufs=2, space="PSUM") as ps_t, \
             tc.tile_pool(name="ps_v", bufs=2, space="PSUM") as ps_v, \
             tc.tile_pool(name="ps_y", bufs=3, space="PSUM") as ps_y:
            w8_s = consts.tile([128, NCK, KV], F8, tag="w8")
            x8_s = xs8.tile([128, nslab, NCK, SLAB], F8, tag="x8")
            x16_s = [xs16.tile([128, NCK, VGRP], F16, tag=f"x16_{q}",
                               name=f"x16_{q}") for q in range(ngrp)]
            # sync ring: strict priority order, tiny leading pieces
            # pieces sized for >=4KB per partition line: DMA descriptors
            # are per line, so sub-KB lines are descriptor-rate-bound
            nc.sync.dma_start(w8_s[:, 0:4, :], w8_d[:, 0:4, :])
            nc.sync.dma_start(x8_s[:, 0:2, :, :], x8_d[0])
            nc.sync.dma_start(w8_s[:, 4:8, :], w8_d[:, 4:8, :])
            nc.sync.dma_start(w8_s[:, 8:16, :], w8_d[:, 8:16, :])
            nc.sync.dma_start(x16_s[0][:, 0:8, :], x16_d[0, :, 0:8, :])
            nc.sync.dma_start(x8_s[:, 2:4, :, :], x8_d[1])
            nc.sync.dma_start(x16_s[0][:, 8:16, :], x16_d[0, :, 8:16, :])
            nc.sync.dma_start(x8_s[:, 4:6, :, :], x8_d[2])
            nc.sync.dma_start(x16_s[1][:, 0:8, :], x16_d[1, :, 0:8, :])
            nc.sync.dma_start(x8_s[:, 6:8, :, :], x8_d[3])
            nc.sync.dma_start(x16_s[1][:, 8:16, :], x16_d[1, :, 8:16, :])
            # scalar ring: wf/mt issue AFTER the activation-table preloads
            # below, keeping the first ~3us of DMA bandwidth for the
            # stats-critical sync-ring pieces
            wf_s = consts.tile([128, NCK, D], F16, tag="wf")
            mt_s = consts.tile([128, OUT], F16, tag="mt")
            if with_ba:
                ba_s = consts.tile([1, KV], F8, tag="ba")
                nc.scalar.dma_start(ba_s[:], ba_d)
                ones8_s = consts.tile([1, 128], F8, tag="ones8")
                nc.scalar.dma_start(ones8_s[:], ones8_d)
                vb_s = consts.tile([128, 1], F32, tag="vb")
                nc.scalar.dma_start(vb_s[:], vb_d)
            if with_bv:
                crb_s = consts.tile([128, OUT], F16, tag="crb")
                nc.scalar.dma_start(crb_s[:], crb_d)

            # ---------------- local consts + PE warm-up ----------------
            eps_s = consts.tile([128, 1], F32, tag="eps")
            nc.vector.memset(eps_s[:], EPS)
            jsa = consts.tile([128, 128], F16, tag="jsa")
            nc.vector.memset(jsa[:], 0.0)
            jsb = consts.tile([128, 512], F16, tag="jsb")
            nc.vector.memset(jsb[:], 0.0)
            # preload Square/Sqrt activation tables off the critical path
            tl = consts.tile([128, 1], F32, tag="tl")
            nc.scalar.activation(tl[:], eps_s[:], AF.Square)
            nc.scalar.activation(tl[:], eps_s[:], AF.Sqrt)
            nc.scalar.dma_start(wf_s[:], wf_d)
            nc.scalar.dma_start(mt_s[:], mt_d)
            for i in range(8):
                junk = ps_y.tile([128, 512], F32, tag="py", name=f"junk{i}")
                nc.tensor.matmul(junk[:, 0:256], jsa[:], jsb[:, 0:256],
                                 start=True, stop=True)

            # ---------------- per-stage bodies ----------------
            sq_sc = work.tile([128, KV], F16, tag="sq", bufs=2)
            vts = consts.tile([128, tok], F16, tag="vts")
            s_t = [None] * nslab
            pv_t = [None] * ngrp

            def stats(g):
                pt = ps_t.tile([128, KV], F32, tag="pt", name=f"pt{g}")
                for k in range(8):
                    nc.tensor.matmul(pt[:], x8_s[:, g, 2 * k:2 * k + 2, :],
                                     w8_s[:, 2 * k:2 * k + 2, :],
                                     start=(k == 0),
                                     stop=(k == 7 and not with_ba),
                                     perf_mode=DR)
                if with_ba:
                    nc.tensor.matmul(pt[:], ones8_s[:], ba_s[:],
                                     start=False, stop=True)
                ssq = work.tile([128, 1], F32, tag="ssq")
                nc.scalar.activation(sq_sc[:], pt[:], AF.Square,
                                     accum_out=ssq[:])
                rt = work.tile([128, 1], F32, tag="rt")
                nc.scalar.activation(rt[:], ssq[:], AF.Sqrt, bias=eps_s[:],
                                     scale=1.0 / (KV * WSCALE * WSCALE))
                sg = work.tile([128, 1], F32, tag="sg", name=f"sg{g}", bufs=4)
                nc.vector.reciprocal(sg[:], rt[:])
                s_t[g] = sg

            def vraw_a(q):
                pv = ps_v.tile([128, VGRP], F32, tag="pv", name=f"pv{q}")
                pv_t[q] = pv
                for ck in range(8):
                    nc.tensor.matmul(pv[:], wf_s[:, ck, :], x16_s[q][:, ck, :],
                                     start=(ck == 0), stop=False)

            def vraw_b(q):
                pv = pv_t[q]
                for ck in range(8, NCK):
                    nc.tensor.matmul(pv[:], wf_s[:, ck, :], x16_s[q][:, ck, :],
                                     start=False, stop=(ck == NCK - 1))
                dst = vts[:, q * VGRP:(q + 1) * VGRP]
                if with_ba:
                    nc.scalar.activation(dst, pv[:], AF.Identity,
                                         bias=vb_s[:], scale=1.0)
                else:
                    nc.vector.tensor_copy(dst, pv[:])

            def step4(g):
                t0 = g * SLAB
                ysb = work.tile([128, OUT], F16, tag="ysb", bufs=6)
                for n in range(4):
                    py = ps_y.tile([128, 512], F32, tag="py",
                                   name=f"py{g}_{n}")
                    nc.tensor.matmul(py[:], vts[:, t0:t0 + SLAB],
                                     mt_s[:, n * 512:(n + 1) * 512],
                                     start=True, stop=True)
                    ysl = ysb[:, n * 512:(n + 1) * 512]
                    if n % 2 == 0:
                        nc.vector.tensor_scalar_mul(ysl, py[:], s_t[g][:])
                    else:
                        nc.scalar.activation(ysl, py[:], AF.Identity,
                                             bias=0.0, scale=s_t[g][:])
                    if with_bv:
                        nc.vector.tensor_add(
                            ysl, ysl, crb_s[:, n * 512:(n + 1) * 512])
                    if g == nslab - 1 and n == 1:
                        nc.sync.dma_start(y_d[t0:t0 + SLAB, 0:1024],
                                          ysb[:, 0:1024])
                # alternate output rings: sync's HWDGE is idle once the input
                # issues drain, and two rings halve the serial issue cost
                ring = nc.gpsimd if g % 2 == 0 else nc.sync
                if g == nslab - 1:
                    ring.dma_start(y_d[t0:t0 + SLAB, 1024:2048],
                                   ysb[:, 1024:2048])
                else:
                    ring.dma_start(y_d[t0:t0 + SLAB, :], ysb[:])

            # ---------------- PE emission order ----------------
            stats(0)
            stats(1)
            vraw_a(0)
            vraw_b(0)
            stats(2)
            step4(0)
            stats(3)
            step4(1)
            vraw_a(1)
            stats(4)
            step4(2)
            vraw_b(1)
            stats(5)
            step4(3)
            stats(6)
            step4(4)
            stats(7)
            step4(5)
            step4(6)
            step4(7)

    nc.compile()
    return nc


def _host_prep(inputs):
    h = np.asarray(inputs["hidden_states"], dtype=np.float32)
    b, s, hid = h.shape
    assert hid == HID
    x = np.ascontiguousarray(h.reshape(b * s, hid))
    ntok = b * s
    tok = ntok // N_CORES
    nslab = tok // SLAB
    ngrp = tok // VGRP

    kv_a_w = np.asarray(inputs["kv_a_w"], np.float64)
    kv_a_b = np.asarray(inputs["kv_a_b"], np.float64)
    kv_norm_w = np.asarray(inputs["kv_norm_w"], np.float64)
    kv_b_w = np.asarray(inputs["kv_b_w"], np.float64)
    kv_b_b = np.asarray(inputs["kv_b_b"], np.float64)
    o_w = np.asarray(inputs["o_w"], np.float64)

    wv = kv_b_w[D:2 * D] * (1.0 + kv_norm_w)[None, :]          # (128, 512)
    wf = wv @ kv_a_w                                           # (128, 2048)
    M = o_w.reshape(HID, 16, D).sum(axis=1)                    # (2048, 128)

    w8 = np.ascontiguousarray(
        (kv_a_w.T * WSCALE).reshape(NCK, 128, KV).transpose(1, 0, 2)
    ).astype(np.float32).astype(ml_dtypes.float8_e4m3)
    wf_sw = np.ascontiguousarray(
        wf.T.reshape(NCK, 128, D).transpose(1, 0, 2)).astype(np.float16)
    mt = np.ascontiguousarray(M.T).astype(np.float16)

    with_ba = bool(np.any(kv_a_b != 0.0))
    with_bv = bool(np.any(kv_b_b[D:2 * D] != 0.0))

    in_maps = []
    for i in range(N_CORES):
        shard = x[i * tok:(i + 1) * tok]                       # (tok, 2048)
        xt = shard.T.reshape(NCK, 128, tok)                    # hid-major
        x8 = np.ascontiguousarray(
            xt.reshape(NCK, 128, nslab // 2, 2, SLAB).transpose(2, 1, 3, 0, 4)
        ).astype(ml_dtypes.float8_e4m3)
        x16 = np.ascontiguousarray(
            xt.reshape(NCK, 128, ngrp, VGRP).transpose(2, 1, 0, 3)
        ).astype(np.float16)
        m = {"x8": x8, "x16": x16, "w8": w8, "wf": wf_sw, "mt": mt}
        if with_ba:
            m["bar"] = (kv_a_b.reshape(1, KV) * WSCALE).astype(
                np.float32).astype(ml_dtypes.float8_e4m3)
            m["ones8"] = np.ones((1, 128), np.float32).astype(
                ml_dtypes.float8_e4m3)
            m["vb"] = np.ascontiguousarray(
                (wv @ kv_a_b).reshape(D, 1)).astype(np.float32)
        if with_bv:
            cr = (M @ kv_b_b[D:2 * D]).reshape(1, OUT)
            m["crb"] = np.ascontiguousarray(
                np.broadcast_to(cr, (128, OUT))).astype(np.float16)
        in_maps.append(m)

    def gather(results):
        y = np.concatenate([r["y"] for r in results], axis=0)
        return np.ascontiguousarray(y.reshape(b, s, HID).astype(np.float32))

    return in_maps, gather, with_ba, with_bv, tok


def _run(inputs, trace=False, **spmd_kwargs):
    in_maps, gather, with_ba, with_bv, tok = _host_prep(inputs)
    key = (tok, with_ba, with_bv)
    if key not in _NC_CACHE:
        _NC_CACHE[key] = _build_nc(tok, with_ba, with_bv)
    nc = _NC_CACHE[key]
    res = run_bass_kernel_spmd(nc, in_maps, core_ids=list(range(N_CORES)),
                               trace=trace, **spmd_kwargs)
    return gather(res.results), res


def kernel(**inputs) -> np.ndarray:
    y, _ = _run(inputs, trace=False)
    return y
